# revision 17
# baseline (speedup 1.0000x reference)
"""Trainium2 Bass kernel for leave-one-out Nadaraya-Watson regression
(nn_Net_72877005078649) — fast-Gauss-transform formulation, v2.

Per output channel o this is 1D Gaussian kernel regression; the kernel
factorizes through a G=12 grid (a = b = h/sqrt(2), trapezoid aliasing
~1e-4):  K_h(x,z) ~= kappa * sum_g exp(-(c_g-x)^2/h^2) exp(-(z-c_g)^2/h^2)

v2 design notes (instruction-count-bound on TRN2):
 - host ships transposed/padded layouts: xT/tXT [128(d-pad), n] f32r,
   W1T [128,128], W2rep [128,128] where W2rep[:,p] = W2[p%10,:] — one
   K=128 matmul per 512-col chunk yields XwRep[(g,o)-partition, n]
   directly (no per-tile W2, no transposes, no broadcast ops).
 - source tables in transposed orientation: den[g,o] = sum_n E comes
   free via ACT accum_out on the exp; num via one DVE STT(mult Yrep)
   with accum_out per chunk-pair; YrepT [128, N] = Y[n, p%10] bf16 is
   shipped from host.
 - train side processed as 4 pairs of 512-col chunks ([128,1024] ops).
 - diagonal: train_X == x by construction (the reference's LOO eye-mask
   requires it), so K_ii == 1 exactly: out = (num - Y_d)/(den - 1).
 - query side: Eq[(g,o),b] from the same W2rep path; num/den via one
   K=128 matmul with kappa-and-diagonal-masked tables AA [128,20].

Sharding: queries split across 8 cores (512/core); train replicated.
"""

import numpy as np

N = 4096
D = 64
HID = 128
O = 10
NCORES = 8
BQ = N // NCORES
G = 12
GO = G * O
GRID_LO = -6.5
GRID_HI = 6.5
NPAIR = 4               # train chunk pairs, 1024 cols each

_cache = {}


def _host_consts(h: float):
    c = np.linspace(GRID_LO, GRID_HI, G).astype(np.float32)
    delta = float(c[1] - c[0])
    kappa = 2.0 * delta / (np.sqrt(2.0 * np.pi) * h)
    # consts[128, 21] = cq[128,1] | kmask2[128,20]
    consts = np.zeros((128, 21), np.float32)
    for p in range(128):
        consts[p, 0] = c[min(p // O, G - 1)]
    for p in range(120):
        consts[p, 1 + p % O] = kappa          # num mask
        consts[p, 11 + p % O] = kappa         # den mask
    return consts, kappa


def _host_tensors(x, train_X, Y, W1, W2):
    # transposed, d-padded to 128 partitions
    tXT = np.ascontiguousarray(train_X.T)
    W1T = np.ascontiguousarray(W1.T)
    # W2rep[hid, p] = W2[p%10, hid]
    W2rep = np.empty((HID, 128), np.float32)
    for p in range(128):
        W2rep[:, p] = W2[p % O, :]
    # YrepT[p, n] = Y[n, p%10], bf16
    import jax.numpy as jnp
    Yrep = np.asarray(Y[:, [p % O for p in range(128)]].T)  # [128, N] f32
    Yrep16 = np.asarray(jnp.asarray(Yrep, dtype=jnp.bfloat16))
    return tXT, W1T, W2rep, Yrep16


def _build(h: float):
    import concourse.bass as bass
    import concourse.bacc as bacc
    import concourse.tile as tile
    from concourse import mybir
    from concourse.masks import make_identity

    f32 = mybir.dt.float32
    f32r = mybir.dt.float32r
    bf16 = mybir.dt.bfloat16
    AF = mybir.ActivationFunctionType
    ALU = mybir.AluOpType

    s_n = 1.0 / (h * h)

    nc = bacc.Bacc("TRN2", target_bir_lowering=False, debug=False, num_devices=1)
    xqT = nc.dram_tensor("xqT", [64, BQ], f32r, kind="ExternalInput").ap()
    tXT = nc.dram_tensor("tXT", [64, N], f32r, kind="ExternalInput").ap()
    W1Td = nc.dram_tensor("W1Td", [64, HID], f32r, kind="ExternalInput").ap()
    W2rd = nc.dram_tensor("W2rd", [HID, 128], f32r, kind="ExternalInput").ap()
    Yrd = nc.dram_tensor("Yrd", [128, N], bf16, kind="ExternalInput").ap()
    ydd = nc.dram_tensor("ydd", [BQ, O], f32, kind="ExternalInput").ap()
    constsd = nc.dram_tensor("constsd", [128, 21], f32, kind="ExternalInput").ap()
    out = nc.dram_tensor("out", [BQ, O], f32, kind="ExternalOutput").ap()

    with tile.TileContext(nc) as tc:
        with (
            tc.tile_pool(name="S", bufs=1) as S,
            tc.tile_pool(name="W", bufs=2) as W,
            tc.tile_pool(name="PS", bufs=1, space="PSUM") as PS,
        ):
            # ---- ACT warmup (exp table) ----
            warm = S.tile([1, 16], f32)
            nc.vector.memset(warm, 0.0)
            nc.scalar.activation(out=warm, in_=warm, func=AF.Exp)

            ident = S.tile([128, 128], f32)
            make_identity(nc, ident)

            # ---- input DMAs (issue-engine spread, consumers first) ----
            w1T = S.tile([64, HID], f32r)
            nc.sync.dma_start(out=w1T, in_=W1Td)
            xq_sb = S.tile([64, BQ], f32r)
            nc.sync.dma_start(out=xq_sb, in_=xqT)
            consts = S.tile([128, 21], f32)
            nc.sync.dma_start(out=consts, in_=constsd)
            w2r = S.tile([HID, 128], f32r)
            nc.sync.dma_start(out=w2r, in_=W2rd)
            tX_sb = S.tile([64, N], f32r)
            nc.scalar.dma_start(out=tX_sb[:, 0:2048], in_=tXT[:, 0:2048])
            nc.scalar.dma_start(out=tX_sb[:, 2048:4096], in_=tXT[:, 2048:4096])
            Yr_sb = S.tile([128, N], bf16)
            nc.gpsimd.dma_start(out=Yr_sb, in_=Yrd)
            ydt = S.tile([128, 4 * O], f32)

            cq = consts[:, 0:1]
            kmask2 = consts[:, 1:21]

            nparts = S.tile([128, NPAIR], f32)
            dparts = S.tile([128, NPAIR], f32)
            parts = S.tile([128, 2], f32)
            AA = S.tile([128, 2 * O], f32r)
            Eq = S.tile([128, BQ], f32r)
            dq = S.tile([128, BQ], f32)
            dq2 = S.tile([128, BQ], f32)
            qsb = S.tile([20, BQ], f32)
            nsb = S.tile([128, 4 * O], f32)
            dsb = S.tile([128, 4 * O], f32)
            rsb = S.tile([128, 4 * O], f32)
            osb = S.tile([128, 4 * O], f32)

            fin = PS.tile([128, 128], f32, tag="fin", bufs=1)

            # ---- query chunk first (tail only needs Eq + AA) ----
            hpsq = PS.tile([128, BQ], f32, tag="hq", bufs=1, name="hpsq")
            nc.tensor.matmul(hpsq, lhsT=w1T, rhs=xq_sb, start=True, stop=True)
            h1q = W.tile([128, BQ], f32r, tag="h1q", bufs=1)
            nc.scalar.activation(out=h1q, in_=hpsq, func=AF.Relu)
            xrq = PS.tile([128, BQ], f32, tag="hq", bufs=1, name="xrq")
            nc.tensor.matmul(xrq, lhsT=w2r, rhs=h1q, start=True, stop=True)
            nc.vector.tensor_scalar(out=dq, in0=xrq, scalar1=cq,
                                    scalar2=None, op0=ALU.subtract)
            nc.gpsimd.tensor_tensor(out=dq2, in0=dq, in1=dq, op=ALU.mult)
            nc.scalar.activation(out=Eq, in_=dq2, func=AF.Exp, scale=-s_n)

            # ---- 4 train pairs ----
            for p in range(NPAIR):
                n0 = p * 1024
                hps = PS.tile([128, 1024], f32, tag="hps", bufs=1, name="hps")
                nc.tensor.matmul(hps[:, 0:512], lhsT=w1T,
                                 rhs=tX_sb[:, n0:n0 + 512],
                                 start=True, stop=True)
                nc.tensor.matmul(hps[:, 512:1024], lhsT=w1T,
                                 rhs=tX_sb[:, n0 + 512:n0 + 1024],
                                 start=True, stop=True)
                h1 = W.tile([128, 1024], f32r, tag="h1")
                nc.scalar.activation(out=h1, in_=hps, func=AF.Relu)
                xr = PS.tile([128, 1024], f32, tag="xr", bufs=2, name="xr")
                nc.tensor.matmul(xr[:, 0:512], lhsT=w2r, rhs=h1[:, 0:512],
                                 start=True, stop=True)
                nc.tensor.matmul(xr[:, 512:1024], lhsT=w2r, rhs=h1[:, 512:1024],
                                 start=True, stop=True)
                db = W.tile([128, 1024], f32, tag="db")
                nc.vector.tensor_scalar(out=db, in0=xr, scalar1=cq,
                                        scalar2=None, op0=ALU.subtract)
                d2 = W.tile([128, 1024], f32, tag="d2")
                nc.gpsimd.tensor_tensor(out=d2, in0=db, in1=db, op=ALU.mult)
                ET = W.tile([128, 1024], bf16, tag="ET")
                nc.scalar.activation(out=ET, in_=d2, func=AF.Exp, scale=-s_n,
                                     accum_out=dparts[:, p:p + 1])
                scr = W.tile([128, 1024], bf16, tag="scr")
                nc.vector.scalar_tensor_tensor(
                    out=scr, in0=ET, scalar=1.0, in1=Yr_sb[:, n0:n0 + 1024],
                    op0=ALU.bypass, op1=ALU.mult,
                    accum_out=nparts[:, p:p + 1])

            nc.sync.dma_start(
                out=ydt.rearrange("p (j o) -> p j o", o=O),
                in_=ydd.rearrange("(j p) o -> p j o", p=128))

            # ---- tables -> AA ----
            nc.vector.tensor_reduce(out=parts[:, 0:1], in_=nparts,
                                    axis=mybir.AxisListType.X, op=ALU.add)
            nc.vector.tensor_reduce(out=parts[:, 1:2], in_=dparts,
                                    axis=mybir.AxisListType.X, op=ALU.add)
            PP = parts.ap[0][0]
            parts_b = bass.AP(tensor=parts.tensor, offset=parts.offset,
                              ap=[[PP, 128], [1, 2], [0, O]])
            nc.vector.tensor_tensor(out=AA.rearrange("p (k e) -> p k e", e=O),
                                    in0=parts_b,
                                    in1=kmask2.rearrange("p (k e) -> p k e", e=O),
                                    op=ALU.mult)

            # ---- query contraction + finalize ----
            qps = PS.tile([20, BQ], f32, tag="hq", bufs=1, name="qps")
            nc.tensor.matmul(qps, lhsT=AA, rhs=Eq, start=True, stop=True)
            nc.scalar.activation(out=qsb, in_=qps, func=AF.Copy)
            for j in range(4):
                nc.tensor.matmul(
                    fin[0:128, j * 20:(j + 1) * 20],
                    lhsT=qsb[:, j * 128:(j + 1) * 128],
                    rhs=ident[0:20, 0:20],
                    is_transpose=True, start=True, stop=True)

            FP = fin.ap[0][0]
            num4 = bass.AP(tensor=fin.tensor, offset=fin.offset,
                           ap=[[FP, 128], [20, 4], [1, O]])
            den4 = bass.AP(tensor=fin.tensor, offset=fin.offset + O,
                           ap=[[FP, 128], [20, 4], [1, O]])
            nc.vector.tensor_tensor(out=nsb, in0=num4, in1=ydt, op=ALU.subtract)
            nc.vector.tensor_scalar(out=dsb, in0=den4, scalar1=-1.0,
                                    scalar2=None, op0=ALU.add)
            nc.vector.reciprocal(rsb, dsb)
            nc.vector.tensor_tensor(out=osb, in0=nsb, in1=rsb, op=ALU.mult)
            nc.sync.dma_start(
                out=out.rearrange("(j p) o -> p j o", p=128),
                in_=osb.rearrange("p (j o) -> p j o", o=O))

    nc.compile()
    return nc


def build_in_maps(x, train_X, Y, W1, W2, h):
    consts, _ = _host_consts(float(h))
    x = np.ascontiguousarray(x, dtype=np.float32)
    train_X = np.ascontiguousarray(train_X, dtype=np.float32)
    Y = np.ascontiguousarray(Y, dtype=np.float32)
    W1 = np.ascontiguousarray(W1, dtype=np.float32)
    W2 = np.ascontiguousarray(W2, dtype=np.float32)
    tXT, W1T, W2rep, Yrep16 = _host_tensors(x, train_X, Y, W1, W2)
    in_maps = []
    for c in range(NCORES):
        sl = slice(c * BQ, (c + 1) * BQ)
        xqT = np.ascontiguousarray(x[sl].T)
        in_maps.append({
            "xqT": xqT, "tXT": tXT, "W1Td": W1T, "W2rd": W2rep,
            "Yrd": Yrep16, "ydd": Y[sl], "constsd": consts,
        })
    return in_maps


def kernel(x, train_X, Y, W1, W2, h):
    import concourse.bass_utils as bass_utils

    hval = float(h)
    key = ("fgt2", hval)
    if key not in _cache:
        _cache[key] = _build(hval)
    nc = _cache[key]

    in_maps = build_in_maps(x, train_X, Y, W1, W2, h)
    res = bass_utils.run_bass_kernel_spmd(nc, in_maps, core_ids=list(range(NCORES)))
    return np.concatenate([res.results[c]["out"] for c in range(NCORES)], axis=0)


# revision 18
# speedup vs baseline: 1.0621x; 1.0621x over previous
"""Trainium2 Bass kernel for leave-one-out Nadaraya-Watson regression
(nn_Net_72877005078649) — fast-Gauss-transform formulation, v2.

Per output channel o this is 1D Gaussian kernel regression; the kernel
factorizes through a G=12 grid (a = b = h/sqrt(2), trapezoid aliasing
~1e-4):  K_h(x,z) ~= kappa * sum_g exp(-(c_g-x)^2/h^2) exp(-(z-c_g)^2/h^2)

v2 design notes (instruction-count-bound on TRN2):
 - host ships transposed/padded layouts: xT/tXT [128(d-pad), n] f32r,
   W1T [128,128], W2rep [128,128] where W2rep[:,p] = W2[p%10,:] — one
   K=128 matmul per 512-col chunk yields XwRep[(g,o)-partition, n]
   directly (no per-tile W2, no transposes, no broadcast ops).
 - source tables in transposed orientation: den[g,o] = sum_n E comes
   free via ACT accum_out on the exp; num via one DVE STT(mult Yrep)
   with accum_out per chunk-pair; YrepT [128, N] = Y[n, p%10] bf16 is
   shipped from host.
 - train side processed as 4 pairs of 512-col chunks ([128,1024] ops).
 - diagonal: train_X == x by construction (the reference's LOO eye-mask
   requires it), so K_ii == 1 exactly: out = (num - Y_d)/(den - 1).
 - query side: Eq[(g,o),b] from the same W2rep path; num/den via one
   K=128 matmul with kappa-and-diagonal-masked tables AA [128,20].

Sharding: queries split across 8 cores (512/core); train replicated.
"""

import numpy as np

N = 4096
D = 64
HID = 128
O = 10
NCORES = 8
BQ = N // NCORES
G = 12
GO = G * O
GRID_LO = -6.5
GRID_HI = 6.5
NPAIR = 4               # train chunk pairs, 1024 cols each

_cache = {}


def _host_consts(h: float):
    c = np.linspace(GRID_LO, GRID_HI, G).astype(np.float32)
    delta = float(c[1] - c[0])
    kappa = 2.0 * delta / (np.sqrt(2.0 * np.pi) * h)
    # consts[128, 21] = cq[128,1] | kmask2[128,20]
    consts = np.zeros((128, 21), np.float32)
    for p in range(128):
        consts[p, 0] = c[min(p // O, G - 1)]
    for p in range(120):
        consts[p, 1 + p % O] = kappa          # num mask
        consts[p, 11 + p % O] = kappa         # den mask
    return consts, kappa


def _host_tensors(x, train_X, Y, W1, W2):
    # transposed, d-padded to 128 partitions
    tXT = np.ascontiguousarray(train_X.T)
    W1T = np.ascontiguousarray(W1.T)
    # W2rep[hid, p] = W2[p%10, hid]
    W2rep = np.empty((HID, 128), np.float32)
    for p in range(128):
        W2rep[:, p] = W2[p % O, :]
    # YrepT[p, n] = Y[n, p%10], bf16
    import jax.numpy as jnp
    Yrep = np.asarray(Y[:, [p % O for p in range(128)]].T)  # [128, N] f32
    Yrep16 = np.asarray(jnp.asarray(Yrep, dtype=jnp.bfloat16))
    return tXT, W1T, W2rep, Yrep16


def _build(h: float):
    import concourse.bass as bass
    import concourse.bacc as bacc
    import concourse.tile as tile
    from concourse import mybir
    from concourse.masks import make_identity

    f32 = mybir.dt.float32
    f32r = mybir.dt.float32r
    bf16 = mybir.dt.bfloat16
    AF = mybir.ActivationFunctionType
    ALU = mybir.AluOpType

    s_n = 1.0 / (h * h)

    nc = bacc.Bacc("TRN2", target_bir_lowering=False, debug=False, num_devices=1)
    xqT = nc.dram_tensor("xqT", [64, BQ], f32r, kind="ExternalInput").ap()
    tXT = nc.dram_tensor("tXT", [64, N], f32r, kind="ExternalInput").ap()
    W1Td = nc.dram_tensor("W1Td", [64, HID], f32r, kind="ExternalInput").ap()
    W2rd = nc.dram_tensor("W2rd", [HID, 128], f32r, kind="ExternalInput").ap()
    Yrd = nc.dram_tensor("Yrd", [128, N], bf16, kind="ExternalInput").ap()
    ydd = nc.dram_tensor("ydd", [BQ, O], f32, kind="ExternalInput").ap()
    constsd = nc.dram_tensor("constsd", [128, 21], f32, kind="ExternalInput").ap()
    out = nc.dram_tensor("out", [BQ, O], f32, kind="ExternalOutput").ap()

    with tile.TileContext(nc) as tc:
        with (
            tc.tile_pool(name="S", bufs=1) as S,
            tc.tile_pool(name="W", bufs=2) as W,
            tc.tile_pool(name="PS", bufs=1, space="PSUM") as PS,
        ):
            # ---- ACT warmup (exp table) ----
            warm = S.tile([1, 16], f32)
            nc.vector.memset(warm, 0.0)
            nc.scalar.activation(out=warm, in_=warm, func=AF.Exp)

            ident = S.tile([128, 128], f32)
            make_identity(nc, ident)

            # ---- input DMAs (issue-engine spread, consumers first) ----
            w1T = S.tile([64, HID], f32r)
            nc.sync.dma_start(out=w1T, in_=W1Td)
            xq_sb = S.tile([64, BQ], f32r)
            nc.sync.dma_start(out=xq_sb, in_=xqT)
            consts = S.tile([128, 21], f32)
            nc.sync.dma_start(out=consts, in_=constsd)
            w2r = S.tile([HID, 128], f32r)
            nc.sync.dma_start(out=w2r, in_=W2rd)
            Yr_sb = S.tile([128, N], bf16)
            nc.scalar.dma_start(out=Yr_sb, in_=Yrd)
            tX_sb = S.tile([64, N], f32r)
            for sl in range(4):
                nc.sync.dma_start(out=tX_sb[:, sl * 1024:(sl + 1) * 1024],
                                  in_=tXT[:, sl * 1024:(sl + 1) * 1024])
            ydt = S.tile([128, 4 * O], f32)

            cq = consts[:, 0:1]
            kmask2 = consts[:, 1:21]

            nparts = S.tile([128, NPAIR], f32)
            dparts = S.tile([128, NPAIR], f32)
            parts = S.tile([128, 2], f32)
            AA = S.tile([128, 2 * O], f32r)
            Eq = S.tile([128, BQ], f32r)
            dq = S.tile([128, BQ], f32)
            dq2 = S.tile([128, BQ], f32)
            qsb = S.tile([20, BQ], f32)
            nsb = S.tile([128, 4 * O], f32)
            dsb = S.tile([128, 4 * O], f32)
            rsb = S.tile([128, 4 * O], f32)
            osb = S.tile([128, 4 * O], f32)

            fin = PS.tile([128, 128], f32, tag="fin", bufs=1)

            # ---- query chunk first (tail only needs Eq + AA) ----
            hpsq = PS.tile([128, BQ], f32, tag="hq", bufs=1, name="hpsq")
            nc.tensor.matmul(hpsq, lhsT=w1T, rhs=xq_sb, start=True, stop=True)
            h1q = W.tile([128, BQ], f32r, tag="h1q", bufs=1)
            nc.scalar.activation(out=h1q, in_=hpsq, func=AF.Relu)
            xrq = PS.tile([128, BQ], f32, tag="hq", bufs=1, name="xrq")
            nc.tensor.matmul(xrq, lhsT=w2r, rhs=h1q, start=True, stop=True)
            nc.vector.tensor_scalar(out=dq, in0=xrq, scalar1=cq,
                                    scalar2=None, op0=ALU.subtract)
            nc.gpsimd.tensor_tensor(out=dq2, in0=dq, in1=dq, op=ALU.mult)
            nc.scalar.activation(out=Eq, in_=dq2, func=AF.Exp, scale=-s_n)

            # ---- 4 train pairs ----
            for p in range(NPAIR):
                n0 = p * 1024
                hps = PS.tile([128, 1024], f32, tag="hps", bufs=1, name="hps")
                nc.tensor.matmul(hps[:, 0:512], lhsT=w1T,
                                 rhs=tX_sb[:, n0:n0 + 512],
                                 start=True, stop=True)
                nc.tensor.matmul(hps[:, 512:1024], lhsT=w1T,
                                 rhs=tX_sb[:, n0 + 512:n0 + 1024],
                                 start=True, stop=True)
                h1 = W.tile([128, 1024], f32r, tag="h1")
                nc.scalar.activation(out=h1, in_=hps, func=AF.Relu)
                xr = PS.tile([128, 1024], f32, tag="xr", bufs=2, name="xr")
                nc.tensor.matmul(xr[:, 0:512], lhsT=w2r, rhs=h1[:, 0:512],
                                 start=True, stop=True)
                nc.tensor.matmul(xr[:, 512:1024], lhsT=w2r, rhs=h1[:, 512:1024],
                                 start=True, stop=True)
                db = W.tile([128, 1024], f32, tag="db")
                nc.vector.tensor_scalar(out=db, in0=xr, scalar1=cq,
                                        scalar2=None, op0=ALU.subtract)
                d2 = W.tile([128, 1024], f32, tag="d2")
                nc.gpsimd.tensor_tensor(out=d2, in0=db, in1=db, op=ALU.mult)
                ET = W.tile([128, 1024], bf16, tag="ET")
                nc.scalar.activation(out=ET, in_=d2, func=AF.Exp, scale=-s_n,
                                     accum_out=dparts[:, p:p + 1])
                scr = W.tile([128, 1024], bf16, tag="scr")
                nc.vector.scalar_tensor_tensor(
                    out=scr, in0=ET, scalar=1.0, in1=Yr_sb[:, n0:n0 + 1024],
                    op0=ALU.bypass, op1=ALU.mult,
                    accum_out=nparts[:, p:p + 1])

            nc.sync.dma_start(
                out=ydt.rearrange("p (j o) -> p j o", o=O),
                in_=ydd.rearrange("(j p) o -> p j o", p=128))

            # ---- tables -> AA ----
            nc.vector.tensor_reduce(out=parts[:, 0:1], in_=nparts,
                                    axis=mybir.AxisListType.X, op=ALU.add)
            nc.vector.tensor_reduce(out=parts[:, 1:2], in_=dparts,
                                    axis=mybir.AxisListType.X, op=ALU.add)
            PP = parts.ap[0][0]
            parts_b = bass.AP(tensor=parts.tensor, offset=parts.offset,
                              ap=[[PP, 128], [1, 2], [0, O]])
            nc.vector.tensor_tensor(out=AA.rearrange("p (k e) -> p k e", e=O),
                                    in0=parts_b,
                                    in1=kmask2.rearrange("p (k e) -> p k e", e=O),
                                    op=ALU.mult)

            # ---- query contraction + finalize ----
            qps = PS.tile([20, BQ], f32, tag="hq", bufs=1, name="qps")
            nc.tensor.matmul(qps, lhsT=AA, rhs=Eq, start=True, stop=True)
            nc.scalar.activation(out=qsb, in_=qps, func=AF.Copy)
            for j in range(4):
                nc.tensor.matmul(
                    fin[0:128, j * 20:(j + 1) * 20],
                    lhsT=qsb[:, j * 128:(j + 1) * 128],
                    rhs=ident[0:20, 0:20],
                    is_transpose=True, start=True, stop=True)

            FP = fin.ap[0][0]
            num4 = bass.AP(tensor=fin.tensor, offset=fin.offset,
                           ap=[[FP, 128], [20, 4], [1, O]])
            den4 = bass.AP(tensor=fin.tensor, offset=fin.offset + O,
                           ap=[[FP, 128], [20, 4], [1, O]])
            nc.vector.tensor_tensor(out=nsb, in0=num4, in1=ydt, op=ALU.subtract)
            nc.vector.tensor_scalar(out=dsb, in0=den4, scalar1=-1.0,
                                    scalar2=None, op0=ALU.add)
            nc.vector.reciprocal(rsb, dsb)
            nc.vector.tensor_tensor(out=osb, in0=nsb, in1=rsb, op=ALU.mult)
            nc.sync.dma_start(
                out=out.rearrange("(j p) o -> p j o", p=128),
                in_=osb.rearrange("p (j o) -> p j o", o=O))

    nc.compile()
    return nc


def build_in_maps(x, train_X, Y, W1, W2, h):
    consts, _ = _host_consts(float(h))
    x = np.ascontiguousarray(x, dtype=np.float32)
    train_X = np.ascontiguousarray(train_X, dtype=np.float32)
    Y = np.ascontiguousarray(Y, dtype=np.float32)
    W1 = np.ascontiguousarray(W1, dtype=np.float32)
    W2 = np.ascontiguousarray(W2, dtype=np.float32)
    tXT, W1T, W2rep, Yrep16 = _host_tensors(x, train_X, Y, W1, W2)
    in_maps = []
    for c in range(NCORES):
        sl = slice(c * BQ, (c + 1) * BQ)
        xqT = np.ascontiguousarray(x[sl].T)
        in_maps.append({
            "xqT": xqT, "tXT": tXT, "W1Td": W1T, "W2rd": W2rep,
            "Yrd": Yrep16, "ydd": Y[sl], "constsd": consts,
        })
    return in_maps


def kernel(x, train_X, Y, W1, W2, h):
    import concourse.bass_utils as bass_utils

    hval = float(h)
    key = ("fgt2", hval)
    if key not in _cache:
        _cache[key] = _build(hval)
    nc = _cache[key]

    in_maps = build_in_maps(x, train_X, Y, W1, W2, h)
    res = bass_utils.run_bass_kernel_spmd(nc, in_maps, core_ids=list(range(NCORES)))
    return np.concatenate([res.results[c]["out"] for c in range(NCORES)], axis=0)


# revision 19
# speedup vs baseline: 1.1362x; 1.0698x over previous
"""Trainium2 Bass kernel for leave-one-out Nadaraya-Watson regression
(nn_Net_72877005078649) — fast-Gauss-transform formulation, v2.

Per output channel o this is 1D Gaussian kernel regression; the kernel
factorizes through a G=12 grid (a = b = h/sqrt(2), trapezoid aliasing
~1e-4):  K_h(x,z) ~= kappa * sum_g exp(-(c_g-x)^2/h^2) exp(-(z-c_g)^2/h^2)

v2 design notes (instruction-count-bound on TRN2):
 - host ships transposed/padded layouts: xT/tXT [128(d-pad), n] f32r,
   W1T [128,128], W2rep [128,128] where W2rep[:,p] = W2[p%10,:] — one
   K=128 matmul per 512-col chunk yields XwRep[(g,o)-partition, n]
   directly (no per-tile W2, no transposes, no broadcast ops).
 - source tables in transposed orientation: den[g,o] = sum_n E comes
   free via ACT accum_out on the exp; num via one DVE STT(mult Yrep)
   with accum_out per chunk-pair; YrepT [128, N] = Y[n, p%10] bf16 is
   shipped from host.
 - train side processed as 4 pairs of 512-col chunks ([128,1024] ops).
 - diagonal: train_X == x by construction (the reference's LOO eye-mask
   requires it), so K_ii == 1 exactly: out = (num - Y_d)/(den - 1).
 - query side: Eq[(g,o),b] from the same W2rep path; num/den via one
   K=128 matmul with kappa-and-diagonal-masked tables AA [128,20].

Sharding: queries split across 8 cores (512/core); train replicated.
"""

import numpy as np

N = 4096
D = 64
HID = 128
O = 10
NCORES = 8
BQ = N // NCORES
G = 12
GO = G * O
GRID_LO = -6.5
GRID_HI = 6.5
NPAIR = 4               # train chunk pairs, 1024 cols each

_cache = {}


def _host_consts(h: float):
    c = np.linspace(GRID_LO, GRID_HI, G).astype(np.float32)
    delta = float(c[1] - c[0])
    kappa = 2.0 * delta / (np.sqrt(2.0 * np.pi) * h)
    # consts[128, 21] = cq[128,1] | kmask2[128,20]
    consts = np.zeros((128, 21), np.float32)
    for p in range(128):
        consts[p, 0] = c[min(p // O, G - 1)]
    for p in range(120):
        consts[p, 1 + p % O] = kappa          # num mask
        consts[p, 11 + p % O] = kappa         # den mask
    return consts, kappa


def _host_tensors(x, train_X, Y, W1, W2):
    # transposed, d-padded to 128 partitions
    tXT = np.ascontiguousarray(train_X.T)
    W1T = np.ascontiguousarray(W1.T)
    # W2rep[hid, p] = W2[p%10, hid]
    W2rep = np.empty((HID, 128), np.float32)
    for p in range(128):
        W2rep[:, p] = W2[p % O, :]
    # YrepT[p, n] = Y[n, p%10], bf16
    import jax.numpy as jnp
    Yrep = np.asarray(Y[:, [p % O for p in range(128)]].T)  # [128, N] f32
    Yrep16 = np.asarray(jnp.asarray(Yrep, dtype=jnp.bfloat16))
    return tXT, W1T, W2rep, Yrep16


def _build(h: float):
    import concourse.bass as bass
    import concourse.bacc as bacc
    import concourse.tile as tile
    from concourse import mybir
    from concourse.masks import make_identity

    f32 = mybir.dt.float32
    f32r = mybir.dt.float32r
    bf16 = mybir.dt.bfloat16
    AF = mybir.ActivationFunctionType
    ALU = mybir.AluOpType

    s_n = 1.0 / (h * h)

    nc = bacc.Bacc("TRN2", target_bir_lowering=False, debug=False, num_devices=1)
    xqT = nc.dram_tensor("xqT", [64, BQ], f32r, kind="ExternalInput").ap()
    tXT = nc.dram_tensor("tXT", [64, N], f32r, kind="ExternalInput").ap()
    W1Td = nc.dram_tensor("W1Td", [64, HID], f32r, kind="ExternalInput").ap()
    W2rd = nc.dram_tensor("W2rd", [HID, 128], f32r, kind="ExternalInput").ap()
    Yrd = nc.dram_tensor("Yrd", [128, N], bf16, kind="ExternalInput").ap()
    ydd = nc.dram_tensor("ydd", [BQ, O], f32, kind="ExternalInput").ap()
    constsd = nc.dram_tensor("constsd", [128, 21], f32, kind="ExternalInput").ap()
    out = nc.dram_tensor("out", [BQ, O], f32, kind="ExternalOutput").ap()

    with tile.TileContext(nc) as tc:
        with (
            tc.tile_pool(name="S", bufs=1) as S,
            tc.tile_pool(name="W", bufs=2) as W,
            tc.tile_pool(name="PS", bufs=1, space="PSUM") as PS,
        ):
            # ---- ACT warmup (exp table) ----
            warm = S.tile([1, 16], f32)
            nc.vector.memset(warm, 0.0)
            nc.scalar.activation(out=warm, in_=warm, func=AF.Exp)

            ident = S.tile([128, 128], f32)
            make_identity(nc, ident)

            # ---- input DMAs: strict priority on the sync queue ----
            w1T = S.tile([64, HID], f32r)
            nc.sync.dma_start(out=w1T, in_=W1Td)
            xq_sb = S.tile([64, BQ], f32r)
            nc.sync.dma_start(out=xq_sb, in_=xqT)
            consts = S.tile([128, 21], f32)
            nc.sync.dma_start(out=consts, in_=constsd)
            w2r = S.tile([HID, 128], f32r)
            nc.sync.dma_start(out=w2r, in_=W2rd)
            ydt = S.tile([128, 4 * O], f32)
            nc.sync.dma_start(
                out=ydt.rearrange("p (j o) -> p j o", o=O),
                in_=ydd.rearrange("(j p) o -> p j o", p=128))
            tX_sb = S.tile([64, N], f32r)
            for sl in range(4):
                nc.sync.dma_start(out=tX_sb[:, sl * 1024:(sl + 1) * 1024],
                                  in_=tXT[:, sl * 1024:(sl + 1) * 1024])
            # Yr on the scalar queue, halves, issued after the warmup so the
            # transfers trail the critical smalls on the fabric
            Yr_sb = S.tile([128, N], bf16)
            nc.scalar.dma_start(out=Yr_sb[:, 0:2048], in_=Yrd[:, 0:2048])
            nc.scalar.dma_start(out=Yr_sb[:, 2048:4096], in_=Yrd[:, 2048:4096])

            cq = consts[:, 0:1]
            kmask2 = consts[:, 1:21]

            nparts = S.tile([128, NPAIR], f32)
            dparts = S.tile([128, NPAIR], f32)
            parts = S.tile([128, 2], f32)
            AA = S.tile([128, 2 * O], f32r)
            Eq = S.tile([128, BQ], f32r)
            dq = S.tile([128, BQ], f32)
            dq2 = S.tile([128, BQ], f32)
            qsb = S.tile([20, BQ], f32)
            nsb = S.tile([128, 4 * O], f32)
            dsb = S.tile([128, 4 * O], f32)
            rsb = S.tile([128, 4 * O], f32)
            osb = S.tile([128, 4 * O], f32)

            # ---- query chunk first (tail only needs Eq + AA) ----
            hpsq = PS.tile([128, 1024], f32, tag="hps", bufs=2, name="hpsq")
            nc.tensor.matmul(hpsq[:, 0:BQ], lhsT=w1T, rhs=xq_sb,
                             start=True, stop=True)
            h1q = W.tile([128, BQ], f32r, tag="h1q", bufs=1)
            nc.scalar.activation(out=h1q, in_=hpsq[:, 0:BQ], func=AF.Relu)
            xrq = PS.tile([128, 1024], f32, tag="xr", bufs=2, name="xrq")
            nc.tensor.matmul(xrq[:, 0:BQ], lhsT=w2r, rhs=h1q,
                             start=True, stop=True)
            nc.vector.tensor_scalar(out=dq, in0=xrq[:, 0:BQ], scalar1=cq,
                                    scalar2=None, op0=ALU.subtract)
            nc.gpsimd.tensor_tensor(out=dq2, in0=dq, in1=dq, op=ALU.mult)
            nc.scalar.activation(out=Eq, in_=dq2, func=AF.Exp, scale=-s_n)

            # ---- 4 train pairs, software-pipelined on PE ----
            hps_t = []
            xr_t = []

            def w1_pair(p):
                n0 = p * 1024
                hps = PS.tile([128, 1024], f32, tag="hps", bufs=2,
                              name=f"hps{p}")
                hps_t.append(hps)
                nc.tensor.matmul(hps[:, 0:512], lhsT=w1T,
                                 rhs=tX_sb[:, n0:n0 + 512],
                                 start=True, stop=True)
                nc.tensor.matmul(hps[:, 512:1024], lhsT=w1T,
                                 rhs=tX_sb[:, n0 + 512:n0 + 1024],
                                 start=True, stop=True)

            w1_pair(0)
            for p in range(NPAIR):
                n0 = p * 1024
                h1 = W.tile([128, 1024], f32r, tag="h1")
                nc.scalar.activation(out=h1, in_=hps_t[p], func=AF.Relu)
                if p + 1 < NPAIR:
                    w1_pair(p + 1)
                xr = PS.tile([128, 1024], f32, tag="xr", bufs=2, name="xr")
                nc.tensor.matmul(xr[:, 0:512], lhsT=w2r, rhs=h1[:, 0:512],
                                 start=True, stop=True)
                nc.tensor.matmul(xr[:, 512:1024], lhsT=w2r, rhs=h1[:, 512:1024],
                                 start=True, stop=True)
                db = W.tile([128, 1024], f32, tag="db")
                nc.vector.tensor_scalar(out=db, in0=xr, scalar1=cq,
                                        scalar2=None, op0=ALU.subtract)
                d2 = W.tile([128, 1024], f32, tag="d2")
                nc.gpsimd.tensor_tensor(out=d2, in0=db, in1=db, op=ALU.mult)
                ET = W.tile([128, 1024], bf16, tag="ET")
                nc.scalar.activation(out=ET, in_=d2, func=AF.Exp, scale=-s_n,
                                     accum_out=dparts[:, p:p + 1])
                scr = W.tile([128, 1024], bf16, tag="scr")
                nc.vector.scalar_tensor_tensor(
                    out=scr, in0=ET, scalar=1.0, in1=Yr_sb[:, n0:n0 + 1024],
                    op0=ALU.bypass, op1=ALU.mult,
                    accum_out=nparts[:, p:p + 1])

            # ---- tables -> AA ----
            nc.vector.tensor_reduce(out=parts[:, 0:1], in_=nparts,
                                    axis=mybir.AxisListType.X, op=ALU.add)
            nc.vector.tensor_reduce(out=parts[:, 1:2], in_=dparts,
                                    axis=mybir.AxisListType.X, op=ALU.add)
            PP = parts.ap[0][0]
            parts_b = bass.AP(tensor=parts.tensor, offset=parts.offset,
                              ap=[[PP, 128], [1, 2], [0, O]])
            nc.vector.tensor_tensor(out=AA.rearrange("p (k e) -> p k e", e=O),
                                    in0=parts_b,
                                    in1=kmask2.rearrange("p (k e) -> p k e", e=O),
                                    op=ALU.mult)

            # ---- query contraction + finalize ----
            qps_t = PS.tile([128, 1024], f32, tag="xr", bufs=2, name="qps_t")
            qps = qps_t[0:20, 0:BQ]
            nc.tensor.matmul(qps, lhsT=AA, rhs=Eq, start=True, stop=True)
            nc.scalar.activation(out=qsb, in_=qps, func=AF.Copy)
            fin_t = PS.tile([128, 1024], f32, tag="xr", bufs=2, name="fin_t")
            fin = fin_t[:, 0:128]
            for j in range(4):
                nc.tensor.matmul(
                    fin[0:128, j * 20:(j + 1) * 20],
                    lhsT=qsb[:, j * 128:(j + 1) * 128],
                    rhs=ident[0:20, 0:20],
                    is_transpose=True, start=True, stop=True)

            FP = fin.ap[0][0]
            num4 = bass.AP(tensor=fin.tensor, offset=fin.offset,
                           ap=[[FP, 128], [20, 4], [1, O]])
            den4 = bass.AP(tensor=fin.tensor, offset=fin.offset + O,
                           ap=[[FP, 128], [20, 4], [1, O]])
            nc.vector.tensor_tensor(out=nsb, in0=num4, in1=ydt, op=ALU.subtract)
            nc.vector.tensor_scalar(out=dsb, in0=den4, scalar1=-1.0,
                                    scalar2=None, op0=ALU.add)
            nc.vector.reciprocal(rsb, dsb)
            nc.vector.tensor_tensor(out=osb, in0=nsb, in1=rsb, op=ALU.mult)
            nc.sync.dma_start(
                out=out.rearrange("(j p) o -> p j o", p=128),
                in_=osb.rearrange("p (j o) -> p j o", o=O))

    nc.compile()
    return nc


def build_in_maps(x, train_X, Y, W1, W2, h):
    consts, _ = _host_consts(float(h))
    x = np.ascontiguousarray(x, dtype=np.float32)
    train_X = np.ascontiguousarray(train_X, dtype=np.float32)
    Y = np.ascontiguousarray(Y, dtype=np.float32)
    W1 = np.ascontiguousarray(W1, dtype=np.float32)
    W2 = np.ascontiguousarray(W2, dtype=np.float32)
    tXT, W1T, W2rep, Yrep16 = _host_tensors(x, train_X, Y, W1, W2)
    in_maps = []
    for c in range(NCORES):
        sl = slice(c * BQ, (c + 1) * BQ)
        xqT = np.ascontiguousarray(x[sl].T)
        in_maps.append({
            "xqT": xqT, "tXT": tXT, "W1Td": W1T, "W2rd": W2rep,
            "Yrd": Yrep16, "ydd": Y[sl], "constsd": consts,
        })
    return in_maps


def kernel(x, train_X, Y, W1, W2, h):
    import concourse.bass_utils as bass_utils

    hval = float(h)
    key = ("fgt2", hval)
    if key not in _cache:
        _cache[key] = _build(hval)
    nc = _cache[key]

    in_maps = build_in_maps(x, train_X, Y, W1, W2, h)
    res = bass_utils.run_bass_kernel_spmd(nc, in_maps, core_ids=list(range(NCORES)))
    return np.concatenate([res.results[c]["out"] for c in range(NCORES)], axis=0)


# revision 20
# speedup vs baseline: 1.1718x; 1.0313x over previous
"""Trainium2 Bass kernel for leave-one-out Nadaraya-Watson regression
(nn_Net_72877005078649) — fast-Gauss-transform formulation, v2.

Per output channel o this is 1D Gaussian kernel regression; the kernel
factorizes through a G=12 grid (a = b = h/sqrt(2), trapezoid aliasing
~1e-4):  K_h(x,z) ~= kappa * sum_g exp(-(c_g-x)^2/h^2) exp(-(z-c_g)^2/h^2)

v2 design notes (instruction-count-bound on TRN2):
 - host ships transposed/padded layouts: xT/tXT [128(d-pad), n] f32r,
   W1T [128,128], W2rep [128,128] where W2rep[:,p] = W2[p%10,:] — one
   K=128 matmul per 512-col chunk yields XwRep[(g,o)-partition, n]
   directly (no per-tile W2, no transposes, no broadcast ops).
 - source tables in transposed orientation: den[g,o] = sum_n E comes
   free via ACT accum_out on the exp; num via one DVE STT(mult Yrep)
   with accum_out per chunk-pair; YrepT [128, N] = Y[n, p%10] bf16 is
   shipped from host.
 - train side processed as 4 pairs of 512-col chunks ([128,1024] ops).
 - diagonal: train_X == x by construction (the reference's LOO eye-mask
   requires it), so K_ii == 1 exactly: out = (num - Y_d)/(den - 1).
 - query side: Eq[(g,o),b] from the same W2rep path; num/den via one
   K=128 matmul with kappa-and-diagonal-masked tables AA [128,20].

Sharding: queries split across 8 cores (512/core); train replicated.
"""

import numpy as np

N = 4096
D = 64
HID = 128
O = 10
NCORES = 8
BQ = N // NCORES
G = 12
GO = G * O
GRID_LO = -6.5
GRID_HI = 6.5
NPAIR = 4               # train chunk pairs, 1024 cols each

_cache = {}


def _host_consts(h: float):
    c = np.linspace(GRID_LO, GRID_HI, G).astype(np.float32)
    delta = float(c[1] - c[0])
    kappa = 2.0 * delta / (np.sqrt(2.0 * np.pi) * h)
    # consts[128, 21] = cq[128,1] | kmask2[128,20]
    consts = np.zeros((128, 21), np.float32)
    for p in range(128):
        consts[p, 0] = c[min(p // O, G - 1)]
    for p in range(120):
        consts[p, 1 + p % O] = kappa          # num mask
        consts[p, 11 + p % O] = kappa         # den mask
    return consts, kappa


def _host_tensors(x, train_X, Y, W1, W2):
    # transposed, d-padded to 128 partitions
    import jax.numpy as jnp
    tXT = np.asarray(jnp.asarray(train_X.T, dtype=jnp.bfloat16))
    W1T = np.asarray(jnp.asarray(W1.T, dtype=jnp.bfloat16))
    # W2rep[hid, p] = W2[p%10, hid]
    W2rep = np.empty((HID, 128), np.float32)
    for p in range(128):
        W2rep[:, p] = W2[p % O, :]
    # YrepT[p, n] = Y[n, p%10], bf16
    Yrep = np.asarray(Y[:, [p % O for p in range(128)]].T)  # [128, N] f32
    Yrep16 = np.asarray(jnp.asarray(Yrep, dtype=jnp.bfloat16))
    return tXT, W1T, W2rep, Yrep16


def _build(h: float):
    import concourse.bass as bass
    import concourse.bacc as bacc
    import concourse.tile as tile
    from concourse import mybir
    from concourse.masks import make_identity

    f32 = mybir.dt.float32
    f32r = mybir.dt.float32r
    bf16 = mybir.dt.bfloat16
    AF = mybir.ActivationFunctionType
    ALU = mybir.AluOpType

    s_n = 1.0 / (h * h)

    nc = bacc.Bacc("TRN2", target_bir_lowering=False, debug=False, num_devices=1)
    xqT = nc.dram_tensor("xqT", [64, BQ], bf16, kind="ExternalInput").ap()
    tXT = nc.dram_tensor("tXT", [64, N], bf16, kind="ExternalInput").ap()
    W1Td = nc.dram_tensor("W1Td", [64, HID], bf16, kind="ExternalInput").ap()
    W2rd = nc.dram_tensor("W2rd", [HID, 128], f32r, kind="ExternalInput").ap()
    Yrd = nc.dram_tensor("Yrd", [128, N], bf16, kind="ExternalInput").ap()
    ydd = nc.dram_tensor("ydd", [BQ, O], f32, kind="ExternalInput").ap()
    constsd = nc.dram_tensor("constsd", [128, 21], f32, kind="ExternalInput").ap()
    out = nc.dram_tensor("out", [BQ, O], f32, kind="ExternalOutput").ap()

    with tile.TileContext(nc) as tc:
        with (
            tc.tile_pool(name="S", bufs=1) as S,
            tc.tile_pool(name="W", bufs=2) as W,
            tc.tile_pool(name="PS", bufs=1, space="PSUM") as PS,
        ):
            # ---- ACT warmup (exp table) ----
            warm = S.tile([1, 16], f32)
            nc.vector.memset(warm, 0.0)
            nc.scalar.activation(out=warm, in_=warm, func=AF.Exp)

            ident = S.tile([128, 128], f32)
            make_identity(nc, ident)

            # ---- input DMAs: strict priority on the sync queue ----
            w1T = S.tile([64, HID], bf16)
            nc.sync.dma_start(out=w1T, in_=W1Td)
            xq_sb = S.tile([64, BQ], bf16)
            nc.sync.dma_start(out=xq_sb, in_=xqT)
            consts = S.tile([128, 21], f32)
            nc.sync.dma_start(out=consts, in_=constsd)
            w2r = S.tile([HID, 128], f32r)
            nc.sync.dma_start(out=w2r, in_=W2rd)
            ydt = S.tile([128, 4 * O], f32)
            nc.sync.dma_start(
                out=ydt.rearrange("p (j o) -> p j o", o=O),
                in_=ydd.rearrange("(j p) o -> p j o", p=128))
            tX_sb = S.tile([64, N], bf16)
            for sl in range(4):
                nc.sync.dma_start(out=tX_sb[:, sl * 1024:(sl + 1) * 1024],
                                  in_=tXT[:, sl * 1024:(sl + 1) * 1024])
            # Yr on the scalar queue, halves, issued after the warmup so the
            # transfers trail the critical smalls on the fabric
            Yr_sb = S.tile([128, N], bf16)
            nc.scalar.dma_start(out=Yr_sb[:, 0:2048], in_=Yrd[:, 0:2048])
            nc.scalar.dma_start(out=Yr_sb[:, 2048:4096], in_=Yrd[:, 2048:4096])

            cq = consts[:, 0:1]
            kmask2 = consts[:, 1:21]

            nparts = S.tile([128, NPAIR], f32)
            dparts = S.tile([128, NPAIR], f32)
            parts = S.tile([128, 2], f32)
            AA = S.tile([128, 2 * O], f32r)
            Eq = S.tile([128, BQ], f32r)
            dq = S.tile([128, BQ], f32)
            dq2 = S.tile([128, BQ], f32)
            qsb = S.tile([20, BQ], f32)
            nsb = S.tile([128, 4 * O], f32)
            dsb = S.tile([128, 4 * O], f32)
            rsb = S.tile([128, 4 * O], f32)
            osb = S.tile([128, 4 * O], f32)

            # ---- query chunk first (tail only needs Eq + AA) ----
            hpsq = PS.tile([128, 1024], f32, tag="hps", bufs=2, name="hpsq")
            nc.tensor.matmul(hpsq[:, 0:BQ], lhsT=w1T, rhs=xq_sb,
                             start=True, stop=True)
            h1q = W.tile([128, BQ], f32r, tag="h1q", bufs=1)
            nc.scalar.activation(out=h1q, in_=hpsq[:, 0:BQ], func=AF.Relu)
            xrq = PS.tile([128, 1024], f32, tag="xr", bufs=2, name="xrq")
            nc.tensor.matmul(xrq[:, 0:BQ], lhsT=w2r, rhs=h1q,
                             start=True, stop=True)
            nc.vector.tensor_scalar(out=dq, in0=xrq[:, 0:BQ], scalar1=cq,
                                    scalar2=None, op0=ALU.subtract)
            nc.gpsimd.tensor_tensor(out=dq2, in0=dq, in1=dq, op=ALU.mult)
            nc.scalar.activation(out=Eq, in_=dq2, func=AF.Exp, scale=-s_n)

            # ---- 4 train pairs, software-pipelined on PE ----
            hps_t = []
            xr_t = []

            def w1_pair(p):
                n0 = p * 1024
                hps = PS.tile([128, 1024], f32, tag="hps", bufs=2,
                              name=f"hps{p}")
                hps_t.append(hps)
                nc.tensor.matmul(hps[:, 0:512], lhsT=w1T,
                                 rhs=tX_sb[:, n0:n0 + 512],
                                 start=True, stop=True)
                nc.tensor.matmul(hps[:, 512:1024], lhsT=w1T,
                                 rhs=tX_sb[:, n0 + 512:n0 + 1024],
                                 start=True, stop=True)

            w1_pair(0)
            for p in range(NPAIR):
                n0 = p * 1024
                h1 = W.tile([128, 1024], f32r, tag="h1")
                nc.scalar.activation(out=h1, in_=hps_t[p], func=AF.Relu)
                if p + 1 < NPAIR:
                    w1_pair(p + 1)
                xr = PS.tile([128, 1024], f32, tag="xr", bufs=2, name="xr")
                nc.tensor.matmul(xr[:, 0:512], lhsT=w2r, rhs=h1[:, 0:512],
                                 start=True, stop=True)
                nc.tensor.matmul(xr[:, 512:1024], lhsT=w2r, rhs=h1[:, 512:1024],
                                 start=True, stop=True)
                db = W.tile([128, 1024], f32, tag="db")
                nc.vector.tensor_scalar(out=db, in0=xr, scalar1=cq,
                                        scalar2=None, op0=ALU.subtract)
                d2 = W.tile([128, 1024], f32, tag="d2")
                nc.gpsimd.tensor_tensor(out=d2, in0=db, in1=db, op=ALU.mult)
                ET = W.tile([128, 1024], bf16, tag="ET")
                nc.scalar.activation(out=ET, in_=d2, func=AF.Exp, scale=-s_n,
                                     accum_out=dparts[:, p:p + 1])
                scr = W.tile([128, 1024], bf16, tag="scr")
                nc.vector.scalar_tensor_tensor(
                    out=scr, in0=ET, scalar=1.0, in1=Yr_sb[:, n0:n0 + 1024],
                    op0=ALU.bypass, op1=ALU.mult,
                    accum_out=nparts[:, p:p + 1])

            # ---- tables -> AA ----
            nc.vector.tensor_reduce(out=parts[:, 0:1], in_=nparts,
                                    axis=mybir.AxisListType.X, op=ALU.add)
            nc.vector.tensor_reduce(out=parts[:, 1:2], in_=dparts,
                                    axis=mybir.AxisListType.X, op=ALU.add)
            PP = parts.ap[0][0]
            parts_b = bass.AP(tensor=parts.tensor, offset=parts.offset,
                              ap=[[PP, 128], [1, 2], [0, O]])
            nc.vector.tensor_tensor(out=AA.rearrange("p (k e) -> p k e", e=O),
                                    in0=parts_b,
                                    in1=kmask2.rearrange("p (k e) -> p k e", e=O),
                                    op=ALU.mult)

            # ---- query contraction + finalize ----
            qps_t = PS.tile([128, 1024], f32, tag="xr", bufs=2, name="qps_t")
            qps = qps_t[0:20, 0:BQ]
            nc.tensor.matmul(qps, lhsT=AA, rhs=Eq, start=True, stop=True)
            nc.scalar.activation(out=qsb, in_=qps, func=AF.Copy)
            fin_t = PS.tile([128, 1024], f32, tag="xr", bufs=2, name="fin_t")
            fin = fin_t[:, 0:128]
            for j in range(4):
                nc.tensor.matmul(
                    fin[0:128, j * 20:(j + 1) * 20],
                    lhsT=qsb[:, j * 128:(j + 1) * 128],
                    rhs=ident[0:20, 0:20],
                    is_transpose=True, start=True, stop=True)

            FP = fin.ap[0][0]
            num4 = bass.AP(tensor=fin.tensor, offset=fin.offset,
                           ap=[[FP, 128], [20, 4], [1, O]])
            den4 = bass.AP(tensor=fin.tensor, offset=fin.offset + O,
                           ap=[[FP, 128], [20, 4], [1, O]])
            nc.vector.tensor_tensor(out=nsb, in0=num4, in1=ydt, op=ALU.subtract)
            nc.vector.tensor_scalar(out=dsb, in0=den4, scalar1=-1.0,
                                    scalar2=None, op0=ALU.add)
            nc.vector.reciprocal(rsb, dsb)
            nc.vector.tensor_tensor(out=osb, in0=nsb, in1=rsb, op=ALU.mult)
            nc.sync.dma_start(
                out=out.rearrange("(j p) o -> p j o", p=128),
                in_=osb.rearrange("p (j o) -> p j o", o=O))

    nc.compile()
    return nc


def build_in_maps(x, train_X, Y, W1, W2, h):
    consts, _ = _host_consts(float(h))
    x = np.ascontiguousarray(x, dtype=np.float32)
    train_X = np.ascontiguousarray(train_X, dtype=np.float32)
    Y = np.ascontiguousarray(Y, dtype=np.float32)
    W1 = np.ascontiguousarray(W1, dtype=np.float32)
    W2 = np.ascontiguousarray(W2, dtype=np.float32)
    tXT, W1T, W2rep, Yrep16 = _host_tensors(x, train_X, Y, W1, W2)
    in_maps = []
    for c in range(NCORES):
        sl = slice(c * BQ, (c + 1) * BQ)
        import jax.numpy as jnp
        xqT = np.asarray(jnp.asarray(x[sl].T, dtype=jnp.bfloat16))
        in_maps.append({
            "xqT": xqT, "tXT": tXT, "W1Td": W1T, "W2rd": W2rep,
            "Yrd": Yrep16, "ydd": Y[sl], "constsd": consts,
        })
    return in_maps


def kernel(x, train_X, Y, W1, W2, h):
    import concourse.bass_utils as bass_utils

    hval = float(h)
    key = ("fgt2", hval)
    if key not in _cache:
        _cache[key] = _build(hval)
    nc = _cache[key]

    in_maps = build_in_maps(x, train_X, Y, W1, W2, h)
    res = bass_utils.run_bass_kernel_spmd(nc, in_maps, core_ids=list(range(NCORES)))
    return np.concatenate([res.results[c]["out"] for c in range(NCORES)], axis=0)
